# revision 6
# baseline (speedup 1.0000x reference)
"""Distributed GQA attention (B=2,S=2048,H=2048,NH=16,NKV=4,HD=128) on 8 TRN2 cores.

Strategy: tensor-parallel over heads (2 Q heads + 1 KV head per core) for
QKV-proj + RoPE + causal flash attention, then an AllToAll (2MB/core) to
switch to sequence-parallel for the o_proj (each core computes 512 rows of
the output against the full wo). All matmuls in bf16 (PSUM accumulates f32).
"""

import math

import numpy as np
import ml_dtypes

import concourse.bass as bass
import concourse.mybir as mybir
import concourse.tile as tile
from concourse import bacc
from concourse.bass_utils import run_bass_kernel_spmd
from concourse.masks import make_identity

BF16 = mybir.dt.bfloat16
F32 = mybir.dt.float32

B, S, H = 2, 2048, 2048
NH, NKV, HD = 16, 4, 128
NCORES = 8
HPC = NH // NCORES          # q heads per core = 2
POS = B * S                 # 4096 flattened rows
RPC = POS // NCORES         # output rows per core = 512
KT = H // 128               # 16 contraction tiles for projections
PT_N = POS // 512           # 8 pos-tiles of 512
SCALE = 1.0 / math.sqrt(HD)

_CACHE = {}


def _build():
    nc = bacc.Bacc("TRN2", target_bir_lowering=False, debug=False,
                   num_devices=NCORES)

    xT = nc.declare_dram_parameter("xT", [H, POS], BF16, isOutput=False)
    wq = nc.declare_dram_parameter("wq", [H, HPC * HD], BF16, isOutput=False)
    wk = nc.declare_dram_parameter("wk", [H, HD], BF16, isOutput=False)
    wv = nc.declare_dram_parameter("wv", [H, HD], BF16, isOutput=False)
    cosT = nc.declare_dram_parameter("cosT", [HD, S], F32, isOutput=False)
    ssinT = nc.declare_dram_parameter("ssinT", [HD, S], F32, isOutput=False)
    wo = nc.declare_dram_parameter("wo", [NH * HD, H], BF16, isOutput=False)
    out = nc.declare_dram_parameter("out", [RPC, H], F32, isOutput=True)

    xT_t = xT.ap().rearrange("(k p) n -> p k n", p=128)
    wq_t = wq.ap().rearrange("(k p) m -> p k m", p=128)
    wk_t = wk.ap().rearrange("(k p) m -> p k m", p=128)
    wv_t = wv.ap().rearrange("(k p) m -> p k m", p=128)
    wo_t = wo.ap().rearrange("(k p) m -> p k m", p=128)

    with tile.TileContext(nc) as tc:
        with (
            tc.tile_pool(name="const", bufs=1) as const,
            tc.tile_pool(name="wpool", bufs=1) as wpool,
            tc.tile_pool(name="qkv", bufs=1) as qkv,
            tc.tile_pool(name="dram", bufs=1, space="DRAM") as dram,
        ):
            # ---- constants / weights resident in SBUF ----
            ident = const.tile([128, 128], BF16)
            make_identity(nc, ident)
            # lower-triangular 0/1 mask for the diagonal 128x128 block
            tri = const.tile([128, 128], BF16)
            nc.gpsimd.memset(tri, 1.0)
            nc.gpsimd.affine_select(
                out=tri, in_=tri, compare_op=mybir.AluOpType.is_ge,
                fill=0.0, base=0, pattern=[[-1, 128]], channel_multiplier=1,
            )  # where (p - c) >= 0 keep 1.0 (lower tri), else fill 0.0

            cos_sb = const.tile([128, S], F32)
            sin_sb = const.tile([128, S], F32)
            nc.sync.dma_start(cos_sb[:], cosT.ap())
            nc.sync.dma_start(sin_sb[:], ssinT.ap())

            wq_sb = wpool.tile([128, KT, HPC * HD], BF16)
            wk_sb = wpool.tile([128, KT, HD], BF16)
            wv_sb = wpool.tile([128, KT, HD], BF16)
            nc.sync.dma_start(wq_sb[:], wq_t)
            nc.sync.dma_start(wk_sb[:], wk_t)
            nc.sync.dma_start(wv_sb[:], wv_t)

            # persistent q/k/v for both batches (bf16)
            # qT: [d, pos] per head; kT: [d, pos]; v: [pos-tile, d]
            q_all = qkv.tile([128, HPC, POS], BF16)
            k_all = qkv.tile([128, POS], BF16)
            v_all = qkv.tile([128, POS // 128, HD], BF16)

            a2a_in = dram.tile([NCORES, HPC * HD, RPC], BF16)
            a2a_out = dram.tile([NCORES, HPC * HD, RPC], BF16)

            # ================= Phase 1: QKV projection + RoPE ============
            def rope(dst, ps, c0):
                """dst[128,512] bf16 = ps*cos + swap_halves(ps)*ssin."""
                ra = rope_pool.tile([128, 512], BF16, name="ra", tag="ra",
                                    bufs=3)
                rb = rope_pool.tile([128, 512], BF16, name="rb", tag="rb",
                                    bufs=3)
                nc.vector.tensor_tensor(
                    ra[:], ps[:], cos_sb[:, c0:c0 + 512], mybir.AluOpType.mult)
                nc.vector.tensor_tensor(
                    rb[0:64, :], ps[64:128, :], sin_sb[0:64, c0:c0 + 512],
                    mybir.AluOpType.mult)
                nc.vector.tensor_tensor(
                    rb[64:128, :], ps[0:64, :], sin_sb[64:128, c0:c0 + 512],
                    mybir.AluOpType.mult)
                nc.vector.tensor_tensor(dst, ra[:], rb[:],
                                        mybir.AluOpType.add)

            with (
                tc.tile_pool(name="xtiles", bufs=1) as xtiles,
                tc.tile_pool(name="rope_pool", bufs=1) as rope_pool,
                tc.tile_pool(name="ps1", bufs=1, space="PSUM") as ps1,
            ):
                for pt in range(PT_N):
                    c0 = (pt * 512) % S   # rope table column offset
                    x_t = xtiles.tile([128, KT, 512], BF16, name="x_t",
                                      tag="x", bufs=3)
                    nc.sync.dma_start(x_t[:], xT_t[:, :, pt * 512:(pt + 1) * 512])

                    for hh in range(HPC):
                        ps_q = ps1.tile([128, 512], F32, name="ps_q",
                                        tag="psq", bufs=2)
                        for k in range(KT):
                            nc.tensor.matmul(
                                ps_q[:], wq_sb[:, k, hh * 128:(hh + 1) * 128],
                                x_t[:, k, :], start=(k == 0), stop=(k == KT - 1))
                        rope(q_all[:, hh, pt * 512:(pt + 1) * 512], ps_q, c0)

                    ps_k = ps1.tile([128, 512], F32, name="ps_k", tag="psk",
                                    bufs=2)
                    for k in range(KT):
                        nc.tensor.matmul(ps_k[:], wk_sb[:, k, :], x_t[:, k, :],
                                         start=(k == 0), stop=(k == KT - 1))
                    rope(k_all[:, pt * 512:(pt + 1) * 512], ps_k, c0)

                    for m4 in range(4):
                        ps_v = ps1.tile([128, 128], F32, name="ps_v",
                                        tag="psv", bufs=3)
                        for k in range(KT):
                            nc.tensor.matmul(
                                ps_v[:], x_t[:, k, m4 * 128:(m4 + 1) * 128],
                                wv_sb[:, k, :], start=(k == 0),
                                stop=(k == KT - 1))
                        nc.scalar.copy(v_all[:, pt * 4 + m4, :], ps_v[:])

            # ================= Phase 2: causal flash attention ===========
            with (
                tc.tile_pool(name="att", bufs=1) as att,
                tc.tile_pool(name="ps2", bufs=1, space="PSUM") as ps2,
            ):
                for b in range(B):
                    for hh in range(HPC):
                        qT = q_all[:, hh, b * S:(b + 1) * S]
                        kTb = k_all[:, b * S:(b + 1) * S]
                        voff = b * (S // 128)
                        oT_sb = None
                        for qb in range(S // 128):
                            r = qb % 4
                            nfull = qb // 4      # full 512-wide key tiles
                            width = (qb + 1) * 128
                            p_sb = att.tile([128, S], BF16, name="p_sb",
                                            tag="p", bufs=2)
                            sums = att.tile([128, 8], F32, name="sums",
                                            tag="sums", bufs=2)
                            nslot = 0
                            # full 512-wide key tiles
                            for kt5 in range(nfull):
                                s_ps = ps2.tile([128, 512], F32, name="s_ps",
                                                tag="sps", bufs=2)
                                nc.tensor.matmul(
                                    s_ps[:], qT[:, qb * 128:(qb + 1) * 128],
                                    kTb[:, kt5 * 512:(kt5 + 1) * 512],
                                    start=True, stop=True)
                                nc.scalar.activation(
                                    p_sb[:, kt5 * 512:(kt5 + 1) * 512], s_ps[:],
                                    mybir.ActivationFunctionType.Exp,
                                    scale=SCALE,
                                    accum_out=sums[:, nslot:nslot + 1])
                                nslot += 1
                            # diagonal tile: keys [nfull*512, width)
                            dw = width - nfull * 512   # (r+1)*128
                            s_ps = ps2.tile([128, 512], F32, name="s_psd",
                                            tag="sps", bufs=2)
                            nc.tensor.matmul(
                                s_ps[:, 0:dw], qT[:, qb * 128:(qb + 1) * 128],
                                kTb[:, nfull * 512:width],
                                start=True, stop=True)
                            if r > 0:
                                nc.scalar.activation(
                                    p_sb[:, nfull * 512:nfull * 512 + r * 128],
                                    s_ps[:, 0:r * 128],
                                    mybir.ActivationFunctionType.Exp,
                                    scale=SCALE,
                                    accum_out=sums[:, nslot:nslot + 1])
                                nslot += 1
                            # diagonal 128x128 square: exp then tri-mask
                            nc.scalar.activation(
                                p_sb[:, qb * 128:(qb + 1) * 128],
                                s_ps[:, r * 128:dw],
                                mybir.ActivationFunctionType.Exp, scale=SCALE)
                            nc.vector.tensor_tensor(
                                p_sb[:, qb * 128:(qb + 1) * 128],
                                p_sb[:, qb * 128:(qb + 1) * 128],
                                tri[:], mybir.AluOpType.mult)
                            nc.vector.tensor_reduce(
                                sums[:, nslot:nslot + 1],
                                p_sb[:, qb * 128:(qb + 1) * 128],
                                mybir.AxisListType.X, mybir.AluOpType.add)
                            nslot += 1

                            rsum = att.tile([128, 1], F32, name="rsum",
                                            tag="rsum", bufs=2)
                            recip = att.tile([128, 1], F32, name="recip",
                                             tag="recip", bufs=2)
                            nc.vector.tensor_reduce(
                                rsum[:], sums[:, 0:nslot],
                                mybir.AxisListType.X, mybir.AluOpType.add)
                            nc.vector.reciprocal(recip[:], rsum[:])
                            nc.vector.tensor_scalar_mul(
                                p_sb[:, 0:width], p_sb[:, 0:width], recip[:])

                            # transpose P 128-chunks; PV accumulate
                            o_ps = ps2.tile([128, 128], F32, name="o_ps",
                                            tag="ops", bufs=2)
                            for kc in range(qb + 1):
                                pt_ps = ps2.tile([128, 128], BF16, name="pt_ps",
                                                 tag="ptps", bufs=2)
                                nc.tensor.transpose(
                                    pt_ps[:], p_sb[:, kc * 128:(kc + 1) * 128],
                                    ident[:])
                                pt_sb = att.tile([128, 128], BF16,
                                                 name="pt_sb", tag="ptsb",
                                                 bufs=3)
                                nc.vector.tensor_copy(pt_sb[:], pt_ps[:])
                                nc.tensor.matmul(
                                    o_ps[:], v_all[:, voff + kc, :], pt_sb[:],
                                    start=(kc == 0), stop=(kc == qb))
                            if qb % 4 == 0:
                                oT_sb = att.tile([128, 512], BF16,
                                                 name="oT_sb", tag="osb",
                                                 bufs=2)
                            nc.scalar.copy(
                                oT_sb[:, (qb % 4) * 128:(qb % 4 + 1) * 128],
                                o_ps[:])
                            if qb % 4 == 3:
                                j = b * 4 + qb // 4
                                nc.sync.dma_start(
                                    a2a_in[j, hh * 128:(hh + 1) * 128, :],
                                    oT_sb[:])

            # ================= Phase 3: AllToAll + o_proj ================
            nc.gpsimd.collective_compute(
                "AllToAll", mybir.AluOpType.bypass,
                replica_groups=[list(range(NCORES))],
                ins=[a2a_in.opt()], outs=[a2a_out.opt()])

            with (
                tc.tile_pool(name="proj", bufs=1) as proj,
                tc.tile_pool(name="ps3", bufs=1, space="PSUM") as ps3,
            ):
                wo_sb = proj.tile([128, KT, H], BF16)
                nc.sync.dma_start(wo_sb[:], wo_t)
                at_sb = proj.tile([128, KT, RPC], BF16)
                for k in range(KT):
                    nc.sync.dma_start(
                        at_sb[:, k, :],
                        a2a_out[k // 2, (k % 2) * 128:(k % 2 + 1) * 128, :])
                for mp in range(RPC // 128):
                    for nn in range(H // 512):
                        o_psum = ps3.tile([128, 512], F32, name="o_psum",
                                          tag="po", bufs=3)
                        for k in range(KT):
                            nc.tensor.matmul(
                                o_psum[:], at_sb[:, k, mp * 128:(mp + 1) * 128],
                                wo_sb[:, k, nn * 512:(nn + 1) * 512],
                                start=(k == 0), stop=(k == KT - 1))
                        ev = proj.tile([128, 512], F32, name="ev", tag="ev",
                                       bufs=3)
                        nc.scalar.copy(ev[:], o_psum[:])
                        nc.sync.dma_start(
                            out.ap()[mp * 128:(mp + 1) * 128,
                                     nn * 512:(nn + 1) * 512], ev[:])

    nc.compile()
    return nc


def _get_nc():
    if "nc" not in _CACHE:
        _CACHE["nc"] = _build()
    return _CACHE["nc"]


def _prep_inputs(x, cos, sin, wq, wk, wv, wo):
    bf = ml_dtypes.bfloat16
    xT = np.ascontiguousarray(
        np.asarray(x, np.float32).reshape(POS, H).T).astype(bf)
    cosT = np.ascontiguousarray(np.asarray(cos, np.float32).T)
    sinT = np.asarray(sin, np.float32).T.copy()
    sinT[0:64, :] = -sinT[0:64, :]
    sinT = np.ascontiguousarray(sinT)
    wo_b = np.asarray(wo, np.float32).astype(bf)
    wq = np.asarray(wq, np.float32)
    wk = np.asarray(wk, np.float32)
    wv = np.asarray(wv, np.float32)

    in_maps = []
    for i in range(NCORES):
        kv = i // 2
        in_maps.append({
            "xT": xT,
            "wq": np.ascontiguousarray(
                wq[:, i * HPC * HD:(i + 1) * HPC * HD]).astype(bf),
            "wk": np.ascontiguousarray(
                wk[:, kv * HD:(kv + 1) * HD]).astype(bf),
            "wv": np.ascontiguousarray(
                wv[:, kv * HD:(kv + 1) * HD]).astype(bf),
            "cosT": cosT,
            "ssinT": sinT,
            "wo": wo_b,
        })
    return in_maps


def kernel(x, cos, sin, wq, wk, wv, wo, _trace=False):
    nc = _get_nc()
    in_maps = _prep_inputs(x, cos, sin, wq, wk, wv, wo)
    res = run_bass_kernel_spmd(nc, in_maps, core_ids=list(range(NCORES)),
                               trace=_trace)
    rows = np.concatenate([np.asarray(res.results[i]["out"])
                           for i in range(NCORES)], axis=0)
    out = rows.reshape(B, S, H).astype(np.float32)
    if _trace:
        _CACHE["last_exec_time_ns"] = res.exec_time_ns
        _CACHE["last_results"] = res
    return out


# revision 8
# speedup vs baseline: 1.2935x; 1.2935x over previous
"""Distributed GQA attention (B=2,S=2048,H=2048,NH=16,NKV=4,HD=128) on 8 TRN2 cores.

Strategy: tensor-parallel over heads (2 Q heads + 1 KV head per core) for
QKV-proj + RoPE + causal flash attention, then an AllToAll (2MB/core) to
switch to sequence-parallel for the o_proj (each core computes 512 rows of
the output against the full wo). All matmuls in bf16 (PSUM accumulates f32).
"""

import math

import numpy as np
import ml_dtypes

import concourse.bass as bass
import concourse.mybir as mybir
import concourse.tile as tile
from concourse import bacc
from concourse.bass_utils import run_bass_kernel_spmd
from concourse.masks import make_identity

BF16 = mybir.dt.bfloat16
F32 = mybir.dt.float32

B, S, H = 2, 2048, 2048
NH, NKV, HD = 16, 4, 128
NCORES = 8
HPC = NH // NCORES          # q heads per core = 2
POS = B * S                 # 4096 flattened rows
RPC = POS // NCORES         # output rows per core = 512
KT = H // 128               # 16 contraction tiles for projections
PT_N = POS // 512           # 8 pos-tiles of 512
SCALE = 1.0 / math.sqrt(HD)

_CACHE = {}


def _build():
    nc = bacc.Bacc("TRN2", target_bir_lowering=False, debug=False,
                   num_devices=NCORES)

    xT = nc.declare_dram_parameter("xT", [H, POS], BF16, isOutput=False)
    wq = nc.declare_dram_parameter("wq", [H, HPC * HD], BF16, isOutput=False)
    wk = nc.declare_dram_parameter("wk", [H, HD], BF16, isOutput=False)
    wv = nc.declare_dram_parameter("wv", [H, HD], BF16, isOutput=False)
    cosT = nc.declare_dram_parameter("cosT", [HD, S], F32, isOutput=False)
    ssinT = nc.declare_dram_parameter("ssinT", [HD, S], F32, isOutput=False)
    wo = nc.declare_dram_parameter("wo", [NH * HD, H], BF16, isOutput=False)
    out = nc.declare_dram_parameter("out", [RPC, H], F32, isOutput=True)

    xT_t = xT.ap().rearrange("(k p) n -> p k n", p=128)
    wq_t = wq.ap().rearrange("(k p) m -> p k m", p=128)
    wk_t = wk.ap().rearrange("(k p) m -> p k m", p=128)
    wv_t = wv.ap().rearrange("(k p) m -> p k m", p=128)
    wo_t = wo.ap().rearrange("(k p) m -> p k m", p=128)

    with tile.TileContext(nc) as tc:
        with (
            tc.tile_pool(name="const", bufs=1) as const,
            tc.tile_pool(name="wpool", bufs=1) as wpool,
            tc.tile_pool(name="qkv", bufs=1) as qkv,
            tc.tile_pool(name="dram", bufs=1, space="DRAM") as dram,
        ):
            # ---- constants / weights resident in SBUF ----
            ident = const.tile([128, 128], BF16)
            make_identity(nc, ident)
            # lower-triangular 0/1 mask for the diagonal 128x128 block
            tri = const.tile([128, 128], BF16)
            nc.gpsimd.memset(tri, 1.0)
            nc.gpsimd.affine_select(
                out=tri, in_=tri, compare_op=mybir.AluOpType.is_ge,
                fill=0.0, base=0, pattern=[[-1, 128]], channel_multiplier=1,
            )  # where (p - c) >= 0 keep 1.0 (lower tri), else fill 0.0
            # upper-triangular (incl diag) mask: valid where kpos <= q
            triT = const.tile([128, 128], BF16)
            nc.gpsimd.memset(triT, 1.0)
            nc.gpsimd.affine_select(
                out=triT, in_=triT, compare_op=mybir.AluOpType.is_ge,
                fill=0.0, base=0, pattern=[[1, 128]], channel_multiplier=-1,
            )  # keep 1.0 where (c - p) >= 0, i.e. kpos <= q
            ones_sb = const.tile([128, 128], BF16)
            nc.gpsimd.memset(ones_sb, 1.0)

            cos_sb = const.tile([128, S], F32)
            sin_sb = const.tile([128, S], F32)
            nc.sync.dma_start(cos_sb[:], cosT.ap())
            nc.sync.dma_start(sin_sb[:], ssinT.ap())

            wq_sb = wpool.tile([128, KT, HPC * HD], BF16)
            wk_sb = wpool.tile([128, KT, HD], BF16)
            wv_sb = wpool.tile([128, KT, HD], BF16)
            nc.sync.dma_start(wq_sb[:], wq_t)
            nc.sync.dma_start(wk_sb[:], wk_t)
            nc.sync.dma_start(wv_sb[:], wv_t)

            # persistent q/k/v for both batches (bf16)
            # qT: [d, pos] per head; kT: [d, pos]; v: [pos-tile, d]
            q_all = qkv.tile([128, HPC, POS], BF16)
            k_all = qkv.tile([128, POS], BF16)
            v_all = qkv.tile([128, POS // 128, HD], BF16)

            a2a_in = dram.tile([NCORES, HPC * HD, RPC], BF16)
            a2a_out = dram.tile([NCORES, HPC * HD, RPC], BF16)

            # ================= Phase 1: QKV projection + RoPE ============
            def rope(dst, ps, c0):
                """dst[128,512] bf16 = ps*cos + swap_halves(ps)*ssin."""
                ra = rope_pool.tile([128, 512], BF16, name="ra", tag="ra",
                                    bufs=3)
                rb = rope_pool.tile([128, 512], BF16, name="rb", tag="rb",
                                    bufs=3)
                nc.vector.tensor_tensor(
                    ra[:], ps[:], cos_sb[:, c0:c0 + 512], mybir.AluOpType.mult)
                nc.vector.tensor_tensor(
                    rb[0:64, :], ps[64:128, :], sin_sb[0:64, c0:c0 + 512],
                    mybir.AluOpType.mult)
                nc.vector.tensor_tensor(
                    rb[64:128, :], ps[0:64, :], sin_sb[64:128, c0:c0 + 512],
                    mybir.AluOpType.mult)
                nc.vector.tensor_tensor(dst, ra[:], rb[:],
                                        mybir.AluOpType.add)

            with (
                tc.tile_pool(name="xtiles", bufs=1) as xtiles,
                tc.tile_pool(name="rope_pool", bufs=1) as rope_pool,
                tc.tile_pool(name="ps1", bufs=1, space="PSUM") as ps1,
            ):
                for pt in range(PT_N):
                    c0 = (pt * 512) % S   # rope table column offset
                    x_t = xtiles.tile([128, KT, 512], BF16, name="x_t",
                                      tag="x", bufs=3)
                    nc.sync.dma_start(x_t[:], xT_t[:, :, pt * 512:(pt + 1) * 512])

                    for hh in range(HPC):
                        ps_q = ps1.tile([128, 512], F32, name="ps_q",
                                        tag="psq", bufs=2)
                        for k in range(KT):
                            nc.tensor.matmul(
                                ps_q[:], wq_sb[:, k, hh * 128:(hh + 1) * 128],
                                x_t[:, k, :], start=(k == 0), stop=(k == KT - 1))
                        rope(q_all[:, hh, pt * 512:(pt + 1) * 512], ps_q, c0)

                    ps_k = ps1.tile([128, 512], F32, name="ps_k", tag="psk",
                                    bufs=2)
                    for k in range(KT):
                        nc.tensor.matmul(ps_k[:], wk_sb[:, k, :], x_t[:, k, :],
                                         start=(k == 0), stop=(k == KT - 1))
                    rope(k_all[:, pt * 512:(pt + 1) * 512], ps_k, c0)

                    for m4 in range(4):
                        ps_v = ps1.tile([128, 128], F32, name="ps_v",
                                        tag="psv", bufs=3)
                        for k in range(KT):
                            nc.tensor.matmul(
                                ps_v[:], x_t[:, k, m4 * 128:(m4 + 1) * 128],
                                wv_sb[:, k, :], start=(k == 0),
                                stop=(k == KT - 1))
                        nc.scalar.copy(v_all[:, pt * 4 + m4, :], ps_v[:])

            # ================= Phase 2: causal flash attention ===========
            # ST layout: scores transposed [kpos, q]; exp writes P^T straight
            # to SBUF; denominators via ones-matmul (replicated across
            # partitions); PV consumes P^T directly. 512-query superblocks.
            with (
                tc.tile_pool(name="att", bufs=1) as att,
                tc.tile_pool(name="ps2", bufs=1, space="PSUM") as ps2,
            ):
                for b in range(B):
                    for hh in range(HPC):
                        qT = q_all[:, hh, b * S:(b + 1) * S]
                        kTb = k_all[:, b * S:(b + 1) * S]
                        voff = b * (S // 128)
                        for qsb in range(S // 512):
                            qs = qsb * 512
                            o_ps = ps2.tile([128, 512], F32, name="o_ps",
                                            tag="ops", bufs=2)
                            sum_ps = ps2.tile([128, 512], F32, name="sum_ps",
                                              tag="sums", bufs=2)
                            nkt = 4 * qsb + 4
                            for kt in range(nkt):
                                jj = kt - 4 * qsb   # >=0 on the diagonal
                                c0 = 0 if jj < 0 else jj * 128
                                st_ps = ps2.tile([128, 512], F32, name="st_ps",
                                                 tag="stps", bufs=3)
                                nc.tensor.matmul(
                                    st_ps[:, c0:512],
                                    kTb[:, kt * 128:(kt + 1) * 128],
                                    qT[:, qs + c0:qs + 512],
                                    start=True, stop=True)
                                pt_sb = att.tile([128, 512], BF16,
                                                 name="pt_sb", tag="pt",
                                                 bufs=4)
                                nc.scalar.activation(
                                    pt_sb[:, c0:512], st_ps[:, c0:512],
                                    mybir.ActivationFunctionType.Exp,
                                    scale=SCALE)
                                if jj >= 0:
                                    nc.vector.tensor_tensor(
                                        pt_sb[:, jj * 128:(jj + 1) * 128],
                                        pt_sb[:, jj * 128:(jj + 1) * 128],
                                        triT[:], mybir.AluOpType.mult)
                                nc.tensor.matmul(
                                    sum_ps[:, c0:512], ones_sb[:],
                                    pt_sb[:, c0:512],
                                    start=(kt == 0), stop=(kt == nkt - 1))
                                nc.tensor.matmul(
                                    o_ps[:, c0:512], v_all[:, voff + kt, :],
                                    pt_sb[:, c0:512],
                                    start=(kt == 0), stop=(kt == nkt - 1))

                            recip = att.tile([128, 512], F32, name="recip",
                                             tag="recip", bufs=2)
                            nc.vector.reciprocal(recip[:], sum_ps[:])
                            oT_sb = att.tile([128, 512], BF16, name="oT_sb",
                                             tag="osb", bufs=2)
                            nc.vector.scalar_tensor_tensor(
                                oT_sb[:], o_ps[:], 1.0, recip[:],
                                mybir.AluOpType.mult, mybir.AluOpType.mult)
                            j = b * 4 + qsb
                            nc.sync.dma_start(
                                a2a_in[j, hh * 128:(hh + 1) * 128, :],
                                oT_sb[:])

            # ================= Phase 3: AllToAll + o_proj ================
            nc.gpsimd.collective_compute(
                "AllToAll", mybir.AluOpType.bypass,
                replica_groups=[list(range(NCORES))],
                ins=[a2a_in.opt()], outs=[a2a_out.opt()])

            with (
                tc.tile_pool(name="proj", bufs=1) as proj,
                tc.tile_pool(name="ps3", bufs=1, space="PSUM") as ps3,
            ):
                wo_sb = proj.tile([128, KT, H], BF16)
                nc.sync.dma_start(wo_sb[:], wo_t)
                at_sb = proj.tile([128, KT, RPC], BF16)
                for k in range(KT):
                    nc.sync.dma_start(
                        at_sb[:, k, :],
                        a2a_out[k // 2, (k % 2) * 128:(k % 2 + 1) * 128, :])
                for mp in range(RPC // 128):
                    for nn in range(H // 512):
                        o_psum = ps3.tile([128, 512], F32, name="o_psum",
                                          tag="po", bufs=3)
                        for k in range(KT):
                            nc.tensor.matmul(
                                o_psum[:], at_sb[:, k, mp * 128:(mp + 1) * 128],
                                wo_sb[:, k, nn * 512:(nn + 1) * 512],
                                start=(k == 0), stop=(k == KT - 1))
                        ev = proj.tile([128, 512], F32, name="ev", tag="ev",
                                       bufs=3)
                        nc.scalar.copy(ev[:], o_psum[:])
                        nc.sync.dma_start(
                            out.ap()[mp * 128:(mp + 1) * 128,
                                     nn * 512:(nn + 1) * 512], ev[:])

    nc.compile()
    return nc


def _get_nc():
    if "nc" not in _CACHE:
        _CACHE["nc"] = _build()
    return _CACHE["nc"]


def _prep_inputs(x, cos, sin, wq, wk, wv, wo):
    bf = ml_dtypes.bfloat16
    xT = np.ascontiguousarray(
        np.asarray(x, np.float32).reshape(POS, H).T).astype(bf)
    cosT = np.ascontiguousarray(np.asarray(cos, np.float32).T)
    sinT = np.asarray(sin, np.float32).T.copy()
    sinT[0:64, :] = -sinT[0:64, :]
    sinT = np.ascontiguousarray(sinT)
    wo_b = np.asarray(wo, np.float32).astype(bf)
    wq = np.asarray(wq, np.float32)
    wk = np.asarray(wk, np.float32)
    wv = np.asarray(wv, np.float32)

    in_maps = []
    for i in range(NCORES):
        kv = i // 2
        in_maps.append({
            "xT": xT,
            "wq": np.ascontiguousarray(
                wq[:, i * HPC * HD:(i + 1) * HPC * HD]).astype(bf),
            "wk": np.ascontiguousarray(
                wk[:, kv * HD:(kv + 1) * HD]).astype(bf),
            "wv": np.ascontiguousarray(
                wv[:, kv * HD:(kv + 1) * HD]).astype(bf),
            "cosT": cosT,
            "ssinT": sinT,
            "wo": wo_b,
        })
    return in_maps


def kernel(x, cos, sin, wq, wk, wv, wo, _trace=False):
    nc = _get_nc()
    in_maps = _prep_inputs(x, cos, sin, wq, wk, wv, wo)
    res = run_bass_kernel_spmd(nc, in_maps, core_ids=list(range(NCORES)),
                               trace=_trace)
    rows = np.concatenate([np.asarray(res.results[i]["out"])
                           for i in range(NCORES)], axis=0)
    out = rows.reshape(B, S, H).astype(np.float32)
    if _trace:
        _CACHE["last_exec_time_ns"] = res.exec_time_ns
        _CACHE["last_results"] = res
    return out


# revision 9
# speedup vs baseline: 1.3841x; 1.0700x over previous
"""Distributed GQA attention (B=2,S=2048,H=2048,NH=16,NKV=4,HD=128) on 8 TRN2 cores.

Strategy: tensor-parallel over heads (2 Q heads + 1 KV head per core) for
QKV-proj + RoPE + causal flash attention, then an AllToAll (2MB/core) to
switch to sequence-parallel for the o_proj (each core computes 512 rows of
the output against the full wo). All matmuls in bf16 (PSUM accumulates f32).
"""

import math

import numpy as np
import ml_dtypes

import concourse.bass as bass
import concourse.mybir as mybir
import concourse.tile as tile
from concourse import bacc
from concourse.bass_utils import run_bass_kernel_spmd
from concourse.masks import make_identity

BF16 = mybir.dt.bfloat16
F32 = mybir.dt.float32

B, S, H = 2, 2048, 2048
NH, NKV, HD = 16, 4, 128
NCORES = 8
HPC = NH // NCORES          # q heads per core = 2
POS = B * S                 # 4096 flattened rows
RPC = POS // NCORES         # output rows per core = 512
KT = H // 128               # 16 contraction tiles for projections
PT_N = POS // 512           # 8 pos-tiles of 512
SCALE = 1.0 / math.sqrt(HD)

_CACHE = {}


def _build():
    nc = bacc.Bacc("TRN2", target_bir_lowering=False, debug=False,
                   num_devices=NCORES)

    xT = nc.declare_dram_parameter("xT", [H, POS], BF16, isOutput=False)
    wq = nc.declare_dram_parameter("wq", [H, HPC * HD], BF16, isOutput=False)
    wk = nc.declare_dram_parameter("wk", [H, HD], BF16, isOutput=False)
    wv = nc.declare_dram_parameter("wv", [H, HD], BF16, isOutput=False)
    cosT = nc.declare_dram_parameter("cosT", [HD, S], F32, isOutput=False)
    ssinT = nc.declare_dram_parameter("ssinT", [HD, S], F32, isOutput=False)
    wo = nc.declare_dram_parameter("wo", [NH * HD, H], BF16, isOutput=False)
    out = nc.declare_dram_parameter("out", [RPC, H], F32, isOutput=True)

    xT_t = xT.ap().rearrange("(k p) n -> p k n", p=128)
    wq_t = wq.ap().rearrange("(k p) m -> p k m", p=128)
    wk_t = wk.ap().rearrange("(k p) m -> p k m", p=128)
    wv_t = wv.ap().rearrange("(k p) m -> p k m", p=128)
    wo_t = wo.ap().rearrange("(k p) m -> p k m", p=128)

    with tile.TileContext(nc) as tc:
        with (
            tc.tile_pool(name="const", bufs=1) as const,
            tc.tile_pool(name="wpool", bufs=1) as wpool,
            tc.tile_pool(name="qkv", bufs=1) as qkv,
            tc.tile_pool(name="dram", bufs=1, space="DRAM") as dram,
        ):
            # ---- constants / weights resident in SBUF ----
            ident = const.tile([128, 128], BF16)
            make_identity(nc, ident)
            # lower-triangular 0/1 mask for the diagonal 128x128 block
            tri = const.tile([128, 128], BF16)
            nc.gpsimd.memset(tri, 1.0)
            nc.gpsimd.affine_select(
                out=tri, in_=tri, compare_op=mybir.AluOpType.is_ge,
                fill=0.0, base=0, pattern=[[-1, 128]], channel_multiplier=1,
            )  # where (p - c) >= 0 keep 1.0 (lower tri), else fill 0.0
            # upper-triangular (incl diag) mask: valid where kpos <= q
            triT = const.tile([128, 128], BF16)
            nc.gpsimd.memset(triT, 1.0)
            nc.gpsimd.affine_select(
                out=triT, in_=triT, compare_op=mybir.AluOpType.is_ge,
                fill=0.0, base=0, pattern=[[1, 128]], channel_multiplier=-1,
            )  # keep 1.0 where (c - p) >= 0, i.e. kpos <= q
            ones_sb = const.tile([128, 128], BF16)
            nc.gpsimd.memset(ones_sb, 1.0)

            cos_sb = const.tile([128, S], F32)
            sin_sb = const.tile([128, S], F32)
            nc.sync.dma_start(cos_sb[:], cosT.ap())
            nc.sync.dma_start(sin_sb[:], ssinT.ap())

            wq_sb = wpool.tile([128, KT, HPC * HD], BF16)
            wk_sb = wpool.tile([128, KT, HD], BF16)
            wv_sb = wpool.tile([128, KT, HD], BF16)
            nc.sync.dma_start(wq_sb[:], wq_t)
            nc.sync.dma_start(wk_sb[:], wk_t)
            nc.sync.dma_start(wv_sb[:], wv_t)

            # persistent q/k/v for both batches (bf16)
            # qT: [d, pos] per head; kT: [d, pos]; v: [pos-tile, d]
            q_all = qkv.tile([128, HPC, POS], BF16)
            k_all = qkv.tile([128, POS], BF16)
            v_all = qkv.tile([128, POS // 128, HD], BF16)

            a2a_in1 = dram.tile([NCORES, HD, RPC], BF16)
            a2a_out1 = dram.tile([NCORES, HD, RPC], BF16)
            a2a_in2 = dram.tile([NCORES, HD, RPC], BF16)
            a2a_out2 = dram.tile([NCORES, HD, RPC], BF16)

            # ================= Phase 1: QKV projection + RoPE ============
            def rope(dst, ps, c0):
                """dst[128,512] bf16 = ps*cos + swap_halves(ps)*ssin."""
                ra = rope_pool.tile([128, 512], BF16, name="ra", tag="ra",
                                    bufs=3)
                rb = rope_pool.tile([128, 512], BF16, name="rb", tag="rb",
                                    bufs=3)
                nc.vector.tensor_tensor(
                    ra[:], ps[:], cos_sb[:, c0:c0 + 512], mybir.AluOpType.mult)
                nc.vector.tensor_tensor(
                    rb[0:64, :], ps[64:128, :], sin_sb[0:64, c0:c0 + 512],
                    mybir.AluOpType.mult)
                nc.vector.tensor_tensor(
                    rb[64:128, :], ps[0:64, :], sin_sb[64:128, c0:c0 + 512],
                    mybir.AluOpType.mult)
                nc.vector.tensor_tensor(dst, ra[:], rb[:],
                                        mybir.AluOpType.add)

            with (
                tc.tile_pool(name="xtiles", bufs=1) as xtiles,
                tc.tile_pool(name="rope_pool", bufs=1) as rope_pool,
                tc.tile_pool(name="ps1", bufs=1, space="PSUM") as ps1,
            ):
                for pt in range(PT_N):
                    c0 = (pt * 512) % S   # rope table column offset
                    x_t = xtiles.tile([128, KT, 512], BF16, name="x_t",
                                      tag="x", bufs=3)
                    nc.sync.dma_start(x_t[:], xT_t[:, :, pt * 512:(pt + 1) * 512])

                    for hh in range(HPC):
                        ps_q = ps1.tile([128, 512], F32, name="ps_q",
                                        tag="psq", bufs=2)
                        for k in range(KT):
                            nc.tensor.matmul(
                                ps_q[:], wq_sb[:, k, hh * 128:(hh + 1) * 128],
                                x_t[:, k, :], start=(k == 0), stop=(k == KT - 1))
                        rope(q_all[:, hh, pt * 512:(pt + 1) * 512], ps_q, c0)

                    ps_k = ps1.tile([128, 512], F32, name="ps_k", tag="psk",
                                    bufs=2)
                    for k in range(KT):
                        nc.tensor.matmul(ps_k[:], wk_sb[:, k, :], x_t[:, k, :],
                                         start=(k == 0), stop=(k == KT - 1))
                    rope(k_all[:, pt * 512:(pt + 1) * 512], ps_k, c0)

                    for m4 in range(4):
                        ps_v = ps1.tile([128, 128], F32, name="ps_v",
                                        tag="psv", bufs=3)
                        for k in range(KT):
                            nc.tensor.matmul(
                                ps_v[:], x_t[:, k, m4 * 128:(m4 + 1) * 128],
                                wv_sb[:, k, :], start=(k == 0),
                                stop=(k == KT - 1))
                        nc.scalar.copy(v_all[:, pt * 4 + m4, :], ps_v[:])

            # ================= Phase 2: causal flash attention ===========
            # ST layout: scores transposed [kpos, q]; exp writes P^T straight
            # to SBUF; denominators via ones-matmul (replicated across
            # partitions); PV consumes P^T directly. 512-query superblocks.
            with (
                tc.tile_pool(name="att", bufs=1) as att,
                tc.tile_pool(name="ps2", bufs=1, space="PSUM") as ps2,
            ):
                for hh in range(HPC):
                    a2a_in = a2a_in1 if hh == 0 else a2a_in2
                    a2a_out_h = a2a_out1 if hh == 0 else a2a_out2
                    for b in range(B):
                        qT = q_all[:, hh, b * S:(b + 1) * S]
                        kTb = k_all[:, b * S:(b + 1) * S]
                        voff = b * (S // 128)
                        for qsb in range(S // 512):
                            qs = qsb * 512
                            o_ps = ps2.tile([128, 512], F32, name="o_ps",
                                            tag="ops", bufs=3)
                            sum_ps = ps2.tile([128, 512], F32, name="sum_ps",
                                              tag="sums", bufs=3)
                            nkt = 4 * qsb + 4
                            for kt in range(nkt):
                                jj = kt - 4 * qsb   # >=0 on the diagonal
                                c0 = 0 if jj < 0 else jj * 128
                                st_ps = ps2.tile([128, 512], F32, name="st_ps",
                                                 tag="stps", bufs=2)
                                nc.tensor.matmul(
                                    st_ps[:, c0:512],
                                    kTb[:, kt * 128:(kt + 1) * 128],
                                    qT[:, qs + c0:qs + 512],
                                    start=True, stop=True)
                                pt_sb = att.tile([128, 512], BF16,
                                                 name="pt_sb", tag="pt",
                                                 bufs=4)
                                nc.scalar.activation(
                                    pt_sb[:, c0:512], st_ps[:, c0:512],
                                    mybir.ActivationFunctionType.Exp,
                                    scale=SCALE)
                                if jj >= 0:
                                    nc.vector.tensor_tensor(
                                        pt_sb[:, jj * 128:(jj + 1) * 128],
                                        pt_sb[:, jj * 128:(jj + 1) * 128],
                                        triT[:], mybir.AluOpType.mult)
                                nc.tensor.matmul(
                                    sum_ps[:, c0:512], ones_sb[:],
                                    pt_sb[:, c0:512],
                                    start=(kt == 0), stop=(kt == nkt - 1))
                                nc.tensor.matmul(
                                    o_ps[:, c0:512], v_all[:, voff + kt, :],
                                    pt_sb[:, c0:512],
                                    start=(kt == 0), stop=(kt == nkt - 1))

                            recip = att.tile([128, 512], F32, name="recip",
                                             tag="recip", bufs=2)
                            nc.vector.reciprocal_approx_fast(recip[:],
                                                             sum_ps[:])
                            oT_sb = att.tile([128, 512], BF16, name="oT_sb",
                                             tag="osb", bufs=2)
                            nc.vector.scalar_tensor_tensor(
                                oT_sb[:], o_ps[:], 1.0, recip[:],
                                mybir.AluOpType.mult, mybir.AluOpType.mult)
                            j = b * 4 + qsb
                            nc.sync.dma_start(a2a_in[j, :, :], oT_sb[:])
                    nc.gpsimd.collective_compute(
                        "AllToAll", mybir.AluOpType.bypass,
                        replica_groups=[list(range(NCORES))],
                        ins=[a2a_in.opt()], outs=[a2a_out_h.opt()])

            # ================= Phase 3: o_proj (two-part accumulate) =====
            with (
                tc.tile_pool(name="proj", bufs=1) as proj,
                tc.tile_pool(name="ps3", bufs=1, space="PSUM") as ps3,
            ):
                wo_sb = proj.tile([128, KT, H], BF16)
                nc.sync.dma_start(wo_sb[:], wo_t)
                at1_sb = proj.tile([128, NCORES, RPC], BF16)
                at2_sb = proj.tile([128, NCORES, RPC], BF16)
                for r in range(NCORES):
                    nc.sync.dma_start(at1_sb[:, r, :], a2a_out1[r, :, :])
                for r in range(NCORES):
                    nc.sync.dma_start(at2_sb[:, r, :], a2a_out2[r, :, :])
                for mp in range(RPC // 128):
                    for nn in range(H // 512):
                        o_psum = ps3.tile([128, 512], F32, name="o_psum",
                                          tag="po", bufs=3)
                        for r in range(NCORES):
                            nc.tensor.matmul(
                                o_psum[:], at1_sb[:, r, mp * 128:(mp + 1) * 128],
                                wo_sb[:, 2 * r, nn * 512:(nn + 1) * 512],
                                start=(r == 0), stop=False)
                        for r in range(NCORES):
                            nc.tensor.matmul(
                                o_psum[:], at2_sb[:, r, mp * 128:(mp + 1) * 128],
                                wo_sb[:, 2 * r + 1, nn * 512:(nn + 1) * 512],
                                start=False, stop=(r == NCORES - 1))
                        ev = proj.tile([128, 512], F32, name="ev", tag="ev",
                                       bufs=3)
                        nc.scalar.copy(ev[:], o_psum[:])
                        nc.sync.dma_start(
                            out.ap()[mp * 128:(mp + 1) * 128,
                                     nn * 512:(nn + 1) * 512], ev[:])

    nc.compile()
    return nc


def _get_nc():
    if "nc" not in _CACHE:
        _CACHE["nc"] = _build()
    return _CACHE["nc"]


def _prep_inputs(x, cos, sin, wq, wk, wv, wo):
    bf = ml_dtypes.bfloat16
    xT = np.ascontiguousarray(
        np.asarray(x, np.float32).reshape(POS, H).T).astype(bf)
    cosT = np.ascontiguousarray(np.asarray(cos, np.float32).T)
    sinT = np.asarray(sin, np.float32).T.copy()
    sinT[0:64, :] = -sinT[0:64, :]
    sinT = np.ascontiguousarray(sinT)
    wo_b = np.asarray(wo, np.float32).astype(bf)
    wq = np.asarray(wq, np.float32)
    wk = np.asarray(wk, np.float32)
    wv = np.asarray(wv, np.float32)

    in_maps = []
    for i in range(NCORES):
        kv = i // 2
        in_maps.append({
            "xT": xT,
            "wq": np.ascontiguousarray(
                wq[:, i * HPC * HD:(i + 1) * HPC * HD]).astype(bf),
            "wk": np.ascontiguousarray(
                wk[:, kv * HD:(kv + 1) * HD]).astype(bf),
            "wv": np.ascontiguousarray(
                wv[:, kv * HD:(kv + 1) * HD]).astype(bf),
            "cosT": cosT,
            "ssinT": sinT,
            "wo": wo_b,
        })
    return in_maps


def kernel(x, cos, sin, wq, wk, wv, wo, _trace=False):
    nc = _get_nc()
    in_maps = _prep_inputs(x, cos, sin, wq, wk, wv, wo)
    res = run_bass_kernel_spmd(nc, in_maps, core_ids=list(range(NCORES)),
                               trace=_trace)
    rows = np.concatenate([np.asarray(res.results[i]["out"])
                           for i in range(NCORES)], axis=0)
    out = rows.reshape(B, S, H).astype(np.float32)
    if _trace:
        _CACHE["last_exec_time_ns"] = res.exec_time_ns
        _CACHE["last_results"] = res
    return out


# revision 10
# speedup vs baseline: 1.4216x; 1.0271x over previous
"""Distributed GQA attention (B=2,S=2048,H=2048,NH=16,NKV=4,HD=128) on 8 TRN2 cores.

Strategy: tensor-parallel over heads (2 Q heads + 1 KV head per core) for
QKV-proj + RoPE + causal flash attention, then an AllToAll (2MB/core) to
switch to sequence-parallel for the o_proj (each core computes 512 rows of
the output against the full wo). All matmuls in bf16 (PSUM accumulates f32).
"""

import math

import numpy as np
import ml_dtypes

import concourse.bass as bass
import concourse.mybir as mybir
import concourse.tile as tile
from concourse.tile import add_dep_helper
from concourse import bacc
from concourse.bass_utils import run_bass_kernel_spmd
from concourse.masks import make_identity

BF16 = mybir.dt.bfloat16
F32 = mybir.dt.float32

B, S, H = 2, 2048, 2048
NH, NKV, HD = 16, 4, 128
NCORES = 8
HPC = NH // NCORES          # q heads per core = 2
POS = B * S                 # 4096 flattened rows
RPC = POS // NCORES         # output rows per core = 512
KT = H // 128               # 16 contraction tiles for projections
PT_N = POS // 512           # 8 pos-tiles of 512
SCALE = 1.0 / math.sqrt(HD)

_CACHE = {}


def _build():
    nc = bacc.Bacc("TRN2", target_bir_lowering=False, debug=False,
                   num_devices=NCORES)

    xT = nc.declare_dram_parameter("xT", [H, POS], BF16, isOutput=False)
    wq = nc.declare_dram_parameter("wq", [H, HPC * HD], BF16, isOutput=False)
    wk = nc.declare_dram_parameter("wk", [H, HD], BF16, isOutput=False)
    wv = nc.declare_dram_parameter("wv", [H, HD], BF16, isOutput=False)
    cosT = nc.declare_dram_parameter("cosT", [HD, S], F32, isOutput=False)
    ssinT = nc.declare_dram_parameter("ssinT", [HD, S], F32, isOutput=False)
    wo = nc.declare_dram_parameter("wo", [NH * HD, H], BF16, isOutput=False)
    out = nc.declare_dram_parameter("out", [RPC, H], F32, isOutput=True)

    xT_t = xT.ap().rearrange("(k p) n -> p k n", p=128)
    wq_t = wq.ap().rearrange("(k p) m -> p k m", p=128)
    wk_t = wk.ap().rearrange("(k p) m -> p k m", p=128)
    wv_t = wv.ap().rearrange("(k p) m -> p k m", p=128)
    wo_t = wo.ap().rearrange("(k p) m -> p k m", p=128)

    with tile.TileContext(nc) as tc:
        with (
            tc.tile_pool(name="const", bufs=1) as const,
            tc.tile_pool(name="wpool", bufs=1) as wpool,
            tc.tile_pool(name="qkv", bufs=1) as qkv,
            tc.tile_pool(name="dram", bufs=1, space="DRAM") as dram,
        ):
            # ---- constants / weights resident in SBUF ----
            ident = const.tile([128, 128], BF16)
            make_identity(nc, ident)
            # lower-triangular 0/1 mask for the diagonal 128x128 block
            tri = const.tile([128, 128], BF16)
            nc.gpsimd.memset(tri, 1.0)
            nc.gpsimd.affine_select(
                out=tri, in_=tri, compare_op=mybir.AluOpType.is_ge,
                fill=0.0, base=0, pattern=[[-1, 128]], channel_multiplier=1,
            )  # where (p - c) >= 0 keep 1.0 (lower tri), else fill 0.0
            # upper-triangular (incl diag) mask: valid where kpos <= q
            triT = const.tile([128, 128], BF16)
            nc.gpsimd.memset(triT, 1.0)
            nc.gpsimd.affine_select(
                out=triT, in_=triT, compare_op=mybir.AluOpType.is_ge,
                fill=0.0, base=0, pattern=[[1, 128]], channel_multiplier=-1,
            )  # keep 1.0 where (c - p) >= 0, i.e. kpos <= q
            ones_sb = const.tile([128, 128], BF16)
            nc.gpsimd.memset(ones_sb, 1.0)

            cos_sb = const.tile([128, S], F32)
            sin_sb = const.tile([128, S], F32)
            nc.sync.dma_start(cos_sb[:], cosT.ap())
            nc.sync.dma_start(sin_sb[:], ssinT.ap())

            wq_sb = wpool.tile([128, KT, HPC * HD], BF16)
            wk_sb = wpool.tile([128, KT, HD], BF16)
            wv_sb = wpool.tile([128, KT, HD], BF16)
            nc.sync.dma_start(wq_sb[:], wq_t)
            nc.sync.dma_start(wk_sb[:], wk_t)
            nc.sync.dma_start(wv_sb[:], wv_t)

            # persistent q/k/v for both batches (bf16)
            # qT: [d, pos] per head; kT: [d, pos]; v: [pos-tile, d]
            q_all = qkv.tile([128, HPC, POS], BF16)
            k_all = qkv.tile([128, POS], BF16)
            v_all = qkv.tile([128, POS // 128, HD], BF16)

            a2a_in1 = dram.tile([NCORES, HD, RPC], BF16)
            a2a_out1 = dram.tile([NCORES, HD, RPC], BF16)
            a2a_in2 = dram.tile([NCORES, HD, RPC], BF16)
            a2a_out2 = dram.tile([NCORES, HD, RPC], BF16)

            # ================= Phase 1: QKV projection + RoPE ============
            def rope(dst, ps, c0):
                """dst[128,512] bf16 = ps*cos + swap_halves(ps)*ssin."""
                ra = rope_pool.tile([128, 512], BF16, name="ra", tag="ra",
                                    bufs=3)
                rb = rope_pool.tile([128, 512], BF16, name="rb", tag="rb",
                                    bufs=3)
                nc.vector.tensor_tensor(
                    ra[:], ps[:], cos_sb[:, c0:c0 + 512], mybir.AluOpType.mult)
                nc.vector.tensor_tensor(
                    rb[0:64, :], ps[64:128, :], sin_sb[0:64, c0:c0 + 512],
                    mybir.AluOpType.mult)
                nc.vector.tensor_tensor(
                    rb[64:128, :], ps[0:64, :], sin_sb[64:128, c0:c0 + 512],
                    mybir.AluOpType.mult)
                nc.vector.tensor_tensor(dst, ra[:], rb[:],
                                        mybir.AluOpType.add)

            with (
                tc.tile_pool(name="xtiles", bufs=1) as xtiles,
                tc.tile_pool(name="rope_pool", bufs=1) as rope_pool,
                tc.tile_pool(name="ps1", bufs=1, space="PSUM") as ps1,
            ):
                for pt in range(PT_N):
                    c0 = (pt * 512) % S   # rope table column offset
                    x_t = xtiles.tile([128, KT, 512], BF16, name="x_t",
                                      tag="x", bufs=3)
                    nc.sync.dma_start(x_t[:], xT_t[:, :, pt * 512:(pt + 1) * 512])

                    for hh in range(HPC):
                        ps_q = ps1.tile([128, 512], F32, name="ps_q",
                                        tag="psq", bufs=2)
                        for k in range(KT):
                            nc.tensor.matmul(
                                ps_q[:], wq_sb[:, k, hh * 128:(hh + 1) * 128],
                                x_t[:, k, :], start=(k == 0), stop=(k == KT - 1))
                        rope(q_all[:, hh, pt * 512:(pt + 1) * 512], ps_q, c0)

                    ps_k = ps1.tile([128, 512], F32, name="ps_k", tag="psk",
                                    bufs=2)
                    for k in range(KT):
                        nc.tensor.matmul(ps_k[:], wk_sb[:, k, :], x_t[:, k, :],
                                         start=(k == 0), stop=(k == KT - 1))
                    rope(k_all[:, pt * 512:(pt + 1) * 512], ps_k, c0)

                    for m4 in range(4):
                        ps_v = ps1.tile([128, 128], F32, name="ps_v",
                                        tag="psv", bufs=3)
                        for k in range(KT):
                            nc.tensor.matmul(
                                ps_v[:], x_t[:, k, m4 * 128:(m4 + 1) * 128],
                                wv_sb[:, k, :], start=(k == 0),
                                stop=(k == KT - 1))
                        nc.scalar.copy(v_all[:, pt * 4 + m4, :], ps_v[:])

            # ================= Phase 2: causal flash attention ===========
            # ST layout: scores transposed [kpos, q]; exp writes P^T straight
            # to SBUF; denominators via ones-matmul (replicated across
            # partitions); PV consumes P^T directly. 512-query superblocks.
            with (
                tc.tile_pool(name="att", bufs=1) as att,
                tc.tile_pool(name="ps2", bufs=1, space="PSUM") as ps2,
            ):
                pending = []   # instructions to pull early in the schedule
                for hh in range(HPC):
                    a2a_in = a2a_in1 if hh == 0 else a2a_in2
                    a2a_out_h = a2a_out1 if hh == 0 else a2a_out2
                    for b in range(B):
                        qT = q_all[:, hh, b * S:(b + 1) * S]
                        kTb = k_all[:, b * S:(b + 1) * S]
                        voff = b * (S // 128)
                        for qsb in range(S // 512):
                            qs = qsb * 512
                            o_ps = ps2.tile([128, 512], F32, name="o_ps",
                                            tag="ops", bufs=3)
                            sum_ps = ps2.tile([128, 512], F32, name="sum_ps",
                                              tag="sums", bufs=3)
                            nkt = 4 * qsb + 4
                            for kt in range(nkt):
                                jj = kt - 4 * qsb   # >=0 on the diagonal
                                c0 = 0 if jj < 0 else jj * 128
                                st_ps = ps2.tile([128, 512], F32, name="st_ps",
                                                 tag="stps", bufs=2)
                                mm = nc.tensor.matmul(
                                    st_ps[:, c0:512],
                                    kTb[:, kt * 128:(kt + 1) * 128],
                                    qT[:, qs + c0:qs + 512],
                                    start=True, stop=True)
                                for pend in pending:
                                    add_dep_helper(mm.ins, pend.ins, False)
                                pending = []
                                pt_sb = att.tile([128, 512], BF16,
                                                 name="pt_sb", tag="pt",
                                                 bufs=4)
                                nc.scalar.activation(
                                    pt_sb[:, c0:512], st_ps[:, c0:512],
                                    mybir.ActivationFunctionType.Exp,
                                    scale=SCALE)
                                if jj >= 0:
                                    nc.vector.tensor_tensor(
                                        pt_sb[:, jj * 128:(jj + 1) * 128],
                                        pt_sb[:, jj * 128:(jj + 1) * 128],
                                        triT[:], mybir.AluOpType.mult)
                                nc.tensor.matmul(
                                    sum_ps[:, c0:512], ones_sb[:],
                                    pt_sb[:, c0:512],
                                    start=(kt == 0), stop=(kt == nkt - 1))
                                nc.tensor.matmul(
                                    o_ps[:, c0:512], v_all[:, voff + kt, :],
                                    pt_sb[:, c0:512],
                                    start=(kt == 0), stop=(kt == nkt - 1))

                            recip = att.tile([128, 512], F32, name="recip",
                                             tag="recip", bufs=2)
                            nc.vector.reciprocal_approx_fast(recip[:],
                                                             sum_ps[:])
                            oT_sb = att.tile([128, 512], BF16, name="oT_sb",
                                             tag="osb", bufs=2)
                            nc.vector.scalar_tensor_tensor(
                                oT_sb[:], o_ps[:], 1.0, recip[:],
                                mybir.AluOpType.mult, mybir.AluOpType.mult)
                            j = b * 4 + qsb
                            d = nc.sync.dma_start(a2a_in[j, :, :], oT_sb[:])
                            pending.append(d)
                    cc = nc.gpsimd.collective_compute(
                        "AllToAll", mybir.AluOpType.bypass,
                        replica_groups=[list(range(NCORES))],
                        ins=[a2a_in.opt()], outs=[a2a_out_h.opt()])
                    pending.append(cc)

            # ================= Phase 3: o_proj (two-part accumulate) =====
            with (
                tc.tile_pool(name="proj", bufs=1) as proj,
                tc.tile_pool(name="ps3", bufs=1, space="PSUM") as ps3,
            ):
                wo_sb = proj.tile([128, KT, H], BF16)
                nc.sync.dma_start(wo_sb[:], wo_t)
                at1_sb = proj.tile([128, NCORES, RPC], BF16)
                at2_sb = proj.tile([128, NCORES, RPC], BF16)
                for r in range(NCORES):
                    nc.sync.dma_start(at1_sb[:, r, :], a2a_out1[r, :, :])
                for r in range(NCORES):
                    nc.sync.dma_start(at2_sb[:, r, :], a2a_out2[r, :, :])
                s1_sb = proj.tile([128, 16, 512], F32)
                for mp in range(RPC // 128):
                    for nn in range(H // 512):
                        ti = mp * 4 + nn
                        ps_a = ps3.tile([128, 512], F32, name="ps_a",
                                        tag="po", bufs=4)
                        for r in range(NCORES):
                            nc.tensor.matmul(
                                ps_a[:], at1_sb[:, r, mp * 128:(mp + 1) * 128],
                                wo_sb[:, 2 * r, nn * 512:(nn + 1) * 512],
                                start=(r == 0), stop=(r == NCORES - 1))
                        nc.scalar.copy(s1_sb[:, ti, :], ps_a[:])
                for mp in range(RPC // 128):
                    for nn in range(H // 512):
                        ti = mp * 4 + nn
                        ps_b = ps3.tile([128, 512], F32, name="ps_b",
                                        tag="po", bufs=4)
                        for r in range(NCORES):
                            nc.tensor.matmul(
                                ps_b[:], at2_sb[:, r, mp * 128:(mp + 1) * 128],
                                wo_sb[:, 2 * r + 1, nn * 512:(nn + 1) * 512],
                                start=(r == 0), stop=(r == NCORES - 1))
                        ev = proj.tile([128, 512], F32, name="ev", tag="ev",
                                       bufs=3)
                        nc.vector.scalar_tensor_tensor(
                            ev[:], ps_b[:], 1.0, s1_sb[:, ti, :],
                            mybir.AluOpType.mult, mybir.AluOpType.add)
                        nc.sync.dma_start(
                            out.ap()[mp * 128:(mp + 1) * 128,
                                     nn * 512:(nn + 1) * 512], ev[:])

    nc.compile()
    return nc


def _get_nc():
    if "nc" not in _CACHE:
        _CACHE["nc"] = _build()
    return _CACHE["nc"]


def _prep_inputs(x, cos, sin, wq, wk, wv, wo):
    bf = ml_dtypes.bfloat16
    xT = np.ascontiguousarray(
        np.asarray(x, np.float32).reshape(POS, H).T).astype(bf)
    cosT = np.ascontiguousarray(np.asarray(cos, np.float32).T)
    sinT = np.asarray(sin, np.float32).T.copy()
    sinT[0:64, :] = -sinT[0:64, :]
    sinT = np.ascontiguousarray(sinT)
    wo_b = np.asarray(wo, np.float32).astype(bf)
    wq = np.asarray(wq, np.float32)
    wk = np.asarray(wk, np.float32)
    wv = np.asarray(wv, np.float32)

    in_maps = []
    for i in range(NCORES):
        kv = i // 2
        in_maps.append({
            "xT": xT,
            "wq": np.ascontiguousarray(
                wq[:, i * HPC * HD:(i + 1) * HPC * HD]).astype(bf),
            "wk": np.ascontiguousarray(
                wk[:, kv * HD:(kv + 1) * HD]).astype(bf),
            "wv": np.ascontiguousarray(
                wv[:, kv * HD:(kv + 1) * HD]).astype(bf),
            "cosT": cosT,
            "ssinT": sinT,
            "wo": wo_b,
        })
    return in_maps


def kernel(x, cos, sin, wq, wk, wv, wo, _trace=False):
    nc = _get_nc()
    in_maps = _prep_inputs(x, cos, sin, wq, wk, wv, wo)
    res = run_bass_kernel_spmd(nc, in_maps, core_ids=list(range(NCORES)),
                               trace=_trace)
    rows = np.concatenate([np.asarray(res.results[i]["out"])
                           for i in range(NCORES)], axis=0)
    out = rows.reshape(B, S, H).astype(np.float32)
    if _trace:
        _CACHE["last_exec_time_ns"] = res.exec_time_ns
        _CACHE["last_results"] = res
    return out


# revision 11
# speedup vs baseline: 1.4398x; 1.0128x over previous
"""Distributed GQA attention (B=2,S=2048,H=2048,NH=16,NKV=4,HD=128) on 8 TRN2 cores.

Strategy: tensor-parallel over heads (2 Q heads + 1 KV head per core) for
QKV-proj + RoPE + causal flash attention, then an AllToAll (2MB/core) to
switch to sequence-parallel for the o_proj (each core computes 512 rows of
the output against the full wo). All matmuls in bf16 (PSUM accumulates f32).
"""

import math

import numpy as np
import ml_dtypes

import concourse.bass as bass
import concourse.mybir as mybir
import concourse.tile as tile
from concourse.tile import add_dep_helper
from concourse import bacc
from concourse.bass_utils import run_bass_kernel_spmd
from concourse.masks import make_identity

BF16 = mybir.dt.bfloat16
F32 = mybir.dt.float32

B, S, H = 2, 2048, 2048
NH, NKV, HD = 16, 4, 128
NCORES = 8
HPC = NH // NCORES          # q heads per core = 2
POS = B * S                 # 4096 flattened rows
RPC = POS // NCORES         # output rows per core = 512
KT = H // 128               # 16 contraction tiles for projections
PT_N = POS // 512           # 8 pos-tiles of 512
SCALE = 1.0 / math.sqrt(HD)

_CACHE = {}


def _build():
    nc = bacc.Bacc("TRN2", target_bir_lowering=False, debug=False,
                   num_devices=NCORES)

    xT = nc.declare_dram_parameter("xT", [H, POS], BF16, isOutput=False)
    wq = nc.declare_dram_parameter("wq", [H, HPC * HD], BF16, isOutput=False)
    wk = nc.declare_dram_parameter("wk", [H, HD], BF16, isOutput=False)
    wv = nc.declare_dram_parameter("wv", [H, HD], BF16, isOutput=False)
    cosT = nc.declare_dram_parameter("cosT", [HD, S], BF16, isOutput=False)
    ssinT = nc.declare_dram_parameter("ssinT", [HD, S], BF16, isOutput=False)
    wo = nc.declare_dram_parameter("wo", [NH * HD, H], BF16, isOutput=False)
    out = nc.declare_dram_parameter("out", [RPC, H], F32, isOutput=True)

    xT_t = xT.ap().rearrange("(k p) n -> p k n", p=128)
    wq_t = wq.ap().rearrange("(k p) m -> p k m", p=128)
    wk_t = wk.ap().rearrange("(k p) m -> p k m", p=128)
    wv_t = wv.ap().rearrange("(k p) m -> p k m", p=128)
    wo_t = wo.ap().rearrange("(k p) m -> p k m", p=128)

    with tile.TileContext(nc) as tc:
        with (
            tc.tile_pool(name="const", bufs=1) as const,
            tc.tile_pool(name="wpool", bufs=1) as wpool,
            tc.tile_pool(name="qkv", bufs=1) as qkv,
            tc.tile_pool(name="dram", bufs=1, space="DRAM") as dram,
        ):
            # ---- constants / weights resident in SBUF ----
            ident = const.tile([128, 128], BF16)
            make_identity(nc, ident)
            # lower-triangular 0/1 mask for the diagonal 128x128 block
            tri = const.tile([128, 128], BF16)
            nc.gpsimd.memset(tri, 1.0)
            nc.gpsimd.affine_select(
                out=tri, in_=tri, compare_op=mybir.AluOpType.is_ge,
                fill=0.0, base=0, pattern=[[-1, 128]], channel_multiplier=1,
            )  # where (p - c) >= 0 keep 1.0 (lower tri), else fill 0.0
            # upper-triangular (incl diag) mask: valid where kpos <= q
            triT = const.tile([128, 128], BF16)
            nc.gpsimd.memset(triT, 1.0)
            nc.gpsimd.affine_select(
                out=triT, in_=triT, compare_op=mybir.AluOpType.is_ge,
                fill=0.0, base=0, pattern=[[1, 128]], channel_multiplier=-1,
            )  # keep 1.0 where (c - p) >= 0, i.e. kpos <= q
            ones_sb = const.tile([128, 128], BF16)
            nc.gpsimd.memset(ones_sb, 1.0)

            cos_sb = const.tile([128, S], BF16)
            sin_sb = const.tile([128, S], BF16)
            nc.sync.dma_start(cos_sb[:], cosT.ap())
            nc.sync.dma_start(sin_sb[:], ssinT.ap())

            wq_sb = wpool.tile([128, KT, HPC * HD], BF16)
            wk_sb = wpool.tile([128, KT, HD], BF16)
            wv_sb = wpool.tile([128, KT, HD], BF16)
            nc.sync.dma_start(wq_sb[:], wq_t)
            nc.sync.dma_start(wk_sb[:], wk_t)
            nc.sync.dma_start(wv_sb[:], wv_t)

            # persistent q/k/v for both batches (bf16)
            # qT: [d, pos] per head; kT: [d, pos]; v: [pos-tile, d]
            q_all = qkv.tile([128, HPC, POS], BF16)
            k_all = qkv.tile([128, POS], BF16)
            v_all = qkv.tile([128, POS // 128, HD], BF16)

            a2a_in1 = dram.tile([NCORES, HD, RPC], BF16)
            a2a_out1 = dram.tile([NCORES, HD, RPC], BF16)
            a2a_in2 = dram.tile([NCORES, HD, RPC], BF16)
            a2a_out2 = dram.tile([NCORES, HD, RPC], BF16)

            # ================= Phase 1: QKV projection + RoPE ============
            def rope(dst, ps, c0):
                """dst[128,512] bf16 = ps*cos + swap_halves(ps)*ssin."""
                ra = rope_pool.tile([128, 512], BF16, name="ra", tag="ra",
                                    bufs=3)
                rb = rope_pool.tile([128, 512], BF16, name="rb", tag="rb",
                                    bufs=3)
                nc.vector.tensor_tensor(
                    ra[:], ps[:], cos_sb[:, c0:c0 + 512], mybir.AluOpType.mult)
                nc.vector.tensor_tensor(
                    rb[0:64, :], ps[64:128, :], sin_sb[0:64, c0:c0 + 512],
                    mybir.AluOpType.mult)
                nc.vector.tensor_tensor(
                    rb[64:128, :], ps[0:64, :], sin_sb[64:128, c0:c0 + 512],
                    mybir.AluOpType.mult)
                nc.vector.tensor_tensor(dst, ra[:], rb[:],
                                        mybir.AluOpType.add)

            with (
                tc.tile_pool(name="xtiles", bufs=1) as xtiles,
                tc.tile_pool(name="rope_pool", bufs=1) as rope_pool,
                tc.tile_pool(name="ps1", bufs=1, space="PSUM") as ps1,
            ):
                for pt in range(PT_N):
                    c0 = (pt * 512) % S   # rope table column offset
                    x_t = xtiles.tile([128, KT, 512], BF16, name="x_t",
                                      tag="x", bufs=3)
                    for k4 in range(4):
                        nc.sync.dma_start(
                            x_t[:, k4 * 4:(k4 + 1) * 4, :],
                            xT_t[:, k4 * 4:(k4 + 1) * 4,
                                 pt * 512:(pt + 1) * 512])

                    for hh in range(HPC):
                        ps_q = ps1.tile([128, 512], F32, name="ps_q",
                                        tag="psq", bufs=2)
                        for k in range(KT):
                            nc.tensor.matmul(
                                ps_q[:], wq_sb[:, k, hh * 128:(hh + 1) * 128],
                                x_t[:, k, :], start=(k == 0), stop=(k == KT - 1))
                        rope(q_all[:, hh, pt * 512:(pt + 1) * 512], ps_q, c0)

                    ps_k = ps1.tile([128, 512], F32, name="ps_k", tag="psk",
                                    bufs=2)
                    for k in range(KT):
                        nc.tensor.matmul(ps_k[:], wk_sb[:, k, :], x_t[:, k, :],
                                         start=(k == 0), stop=(k == KT - 1))
                    rope(k_all[:, pt * 512:(pt + 1) * 512], ps_k, c0)

                    for m4 in range(4):
                        ps_v = ps1.tile([128, 128], F32, name="ps_v",
                                        tag="psv", bufs=3)
                        for k in range(KT):
                            nc.tensor.matmul(
                                ps_v[:], x_t[:, k, m4 * 128:(m4 + 1) * 128],
                                wv_sb[:, k, :], start=(k == 0),
                                stop=(k == KT - 1))
                        nc.scalar.copy(v_all[:, pt * 4 + m4, :], ps_v[:])

            # ================= Phase 2: causal flash attention ===========
            # ST layout: scores transposed [kpos, q]; exp writes P^T straight
            # to SBUF; denominators via ones-matmul (replicated across
            # partitions); PV consumes P^T directly. 512-query superblocks.
            with (
                tc.tile_pool(name="att", bufs=1) as att,
                tc.tile_pool(name="ps2", bufs=1, space="PSUM") as ps2,
            ):
                pending = []   # instructions to pull early in the schedule
                first_att_mm = [None]
                for hh in range(HPC):
                    a2a_in = a2a_in1 if hh == 0 else a2a_in2
                    a2a_out_h = a2a_out1 if hh == 0 else a2a_out2
                    for b in range(B):
                        qT = q_all[:, hh, b * S:(b + 1) * S]
                        kTb = k_all[:, b * S:(b + 1) * S]
                        voff = b * (S // 128)
                        for qsb in range(S // 512):
                            qs = qsb * 512
                            o_ps = ps2.tile([128, 512], F32, name="o_ps",
                                            tag="ops", bufs=3)
                            sum_ps = ps2.tile([128, 512], F32, name="sum_ps",
                                              tag="sums", bufs=3)
                            nkt = 4 * qsb + 4
                            for kt in range(nkt):
                                jj = kt - 4 * qsb   # >=0 on the diagonal
                                c0 = 0 if jj < 0 else jj * 128
                                st_ps = ps2.tile([128, 512], F32, name="st_ps",
                                                 tag="stps", bufs=2)
                                mm = nc.tensor.matmul(
                                    st_ps[:, c0:512],
                                    kTb[:, kt * 128:(kt + 1) * 128],
                                    qT[:, qs + c0:qs + 512],
                                    start=True, stop=True)
                                if first_att_mm[0] is None:
                                    first_att_mm[0] = mm
                                for pend in pending:
                                    add_dep_helper(mm.ins, pend.ins, False)
                                pending = []
                                pt_sb = att.tile([128, 512], BF16,
                                                 name="pt_sb", tag="pt",
                                                 bufs=4)
                                nc.scalar.activation(
                                    pt_sb[:, c0:512], st_ps[:, c0:512],
                                    mybir.ActivationFunctionType.Exp,
                                    scale=SCALE)
                                if jj >= 0:
                                    nc.vector.tensor_tensor(
                                        pt_sb[:, jj * 128:(jj + 1) * 128],
                                        pt_sb[:, jj * 128:(jj + 1) * 128],
                                        triT[:], mybir.AluOpType.mult)
                                nc.tensor.matmul(
                                    sum_ps[:, c0:512], ones_sb[:],
                                    pt_sb[:, c0:512],
                                    start=(kt == 0), stop=(kt == nkt - 1))
                                nc.tensor.matmul(
                                    o_ps[:, c0:512], v_all[:, voff + kt, :],
                                    pt_sb[:, c0:512],
                                    start=(kt == 0), stop=(kt == nkt - 1))

                            recip = att.tile([128, 512], F32, name="recip",
                                             tag="recip", bufs=2)
                            nc.vector.reciprocal_approx_fast(recip[:],
                                                             sum_ps[:])
                            oT_sb = att.tile([128, 512], BF16, name="oT_sb",
                                             tag="osb", bufs=2)
                            nc.vector.scalar_tensor_tensor(
                                oT_sb[:], o_ps[:], 1.0, recip[:],
                                mybir.AluOpType.mult, mybir.AluOpType.mult)
                            j = b * 4 + qsb
                            d = nc.sync.dma_start(a2a_in[j, :, :], oT_sb[:])
                            pending.append(d)
                    cc = nc.gpsimd.collective_compute(
                        "AllToAll", mybir.AluOpType.bypass,
                        replica_groups=[list(range(NCORES))],
                        ins=[a2a_in.opt()], outs=[a2a_out_h.opt()])
                    pending.append(cc)

            # ================= Phase 3: o_proj (two-part accumulate) =====
            with (
                tc.tile_pool(name="proj", bufs=1) as proj,
                tc.tile_pool(name="ps3", bufs=1, space="PSUM") as ps3,
            ):
                wo_sb = proj.tile([128, KT, H], BF16)
                for k4 in range(4):
                    wd = nc.sync.dma_start(
                        wo_sb[:, k4 * 4:(k4 + 1) * 4, :],
                        wo_t[:, k4 * 4:(k4 + 1) * 4, :])
                    add_dep_helper(wd.ins, first_att_mm[0].ins, False)
                at1_sb = proj.tile([128, NCORES, RPC], BF16)
                at2_sb = proj.tile([128, NCORES, RPC], BF16)
                for r in range(NCORES):
                    nc.sync.dma_start(at1_sb[:, r, :], a2a_out1[r, :, :])
                for r in range(NCORES):
                    nc.sync.dma_start(at2_sb[:, r, :], a2a_out2[r, :, :])
                s1_sb = proj.tile([128, 16, 512], F32)
                for mp in range(RPC // 128):
                    for nn in range(H // 512):
                        ti = mp * 4 + nn
                        ps_a = ps3.tile([128, 512], F32, name="ps_a",
                                        tag="po", bufs=4)
                        for r in range(NCORES):
                            nc.tensor.matmul(
                                ps_a[:], at1_sb[:, r, mp * 128:(mp + 1) * 128],
                                wo_sb[:, 2 * r, nn * 512:(nn + 1) * 512],
                                start=(r == 0), stop=(r == NCORES - 1))
                        nc.scalar.copy(s1_sb[:, ti, :], ps_a[:])
                for mp in range(RPC // 128):
                    for nn in range(H // 512):
                        ti = mp * 4 + nn
                        ps_b = ps3.tile([128, 512], F32, name="ps_b",
                                        tag="po", bufs=4)
                        for r in range(NCORES):
                            nc.tensor.matmul(
                                ps_b[:], at2_sb[:, r, mp * 128:(mp + 1) * 128],
                                wo_sb[:, 2 * r + 1, nn * 512:(nn + 1) * 512],
                                start=(r == 0), stop=(r == NCORES - 1))
                        ev = proj.tile([128, 512], F32, name="ev", tag="ev",
                                       bufs=3)
                        nc.vector.scalar_tensor_tensor(
                            ev[:], ps_b[:], 1.0, s1_sb[:, ti, :],
                            mybir.AluOpType.mult, mybir.AluOpType.add)
                        nc.sync.dma_start(
                            out.ap()[mp * 128:(mp + 1) * 128,
                                     nn * 512:(nn + 1) * 512], ev[:])

    nc.compile()
    return nc


def _get_nc():
    if "nc" not in _CACHE:
        _CACHE["nc"] = _build()
    return _CACHE["nc"]


def _prep_inputs(x, cos, sin, wq, wk, wv, wo):
    bf = ml_dtypes.bfloat16
    xT = np.ascontiguousarray(
        np.asarray(x, np.float32).reshape(POS, H).T).astype(bf)
    cosT = np.ascontiguousarray(np.asarray(cos, np.float32).T).astype(bf)
    sinT = np.asarray(sin, np.float32).T.copy()
    sinT[0:64, :] = -sinT[0:64, :]
    sinT = np.ascontiguousarray(sinT).astype(bf)
    wo_b = np.asarray(wo, np.float32).astype(bf)
    wq = np.asarray(wq, np.float32)
    wk = np.asarray(wk, np.float32)
    wv = np.asarray(wv, np.float32)

    in_maps = []
    for i in range(NCORES):
        kv = i // 2
        in_maps.append({
            "xT": xT,
            "wq": np.ascontiguousarray(
                wq[:, i * HPC * HD:(i + 1) * HPC * HD]).astype(bf),
            "wk": np.ascontiguousarray(
                wk[:, kv * HD:(kv + 1) * HD]).astype(bf),
            "wv": np.ascontiguousarray(
                wv[:, kv * HD:(kv + 1) * HD]).astype(bf),
            "cosT": cosT,
            "ssinT": sinT,
            "wo": wo_b,
        })
    return in_maps


def kernel(x, cos, sin, wq, wk, wv, wo, _trace=False):
    nc = _get_nc()
    in_maps = _prep_inputs(x, cos, sin, wq, wk, wv, wo)
    res = run_bass_kernel_spmd(nc, in_maps, core_ids=list(range(NCORES)),
                               trace=_trace)
    rows = np.concatenate([np.asarray(res.results[i]["out"])
                           for i in range(NCORES)], axis=0)
    out = rows.reshape(B, S, H).astype(np.float32)
    if _trace:
        _CACHE["last_exec_time_ns"] = res.exec_time_ns
        _CACHE["last_results"] = res
    return out


# revision 12
# speedup vs baseline: 1.4976x; 1.0402x over previous
"""Distributed GQA attention (B=2,S=2048,H=2048,NH=16,NKV=4,HD=128) on 8 TRN2 cores.

Strategy: tensor-parallel over heads (2 Q heads + 1 KV head per core) for
QKV-proj + RoPE + causal flash attention, then an AllToAll (2MB/core) to
switch to sequence-parallel for the o_proj (each core computes 512 rows of
the output against the full wo). All matmuls in bf16 (PSUM accumulates f32).
"""

import math

import numpy as np
import ml_dtypes

import concourse.bass as bass
import concourse.mybir as mybir
import concourse.tile as tile
from concourse.tile import add_dep_helper
from concourse import bacc
from concourse.bass_utils import run_bass_kernel_spmd
from concourse.masks import make_identity

BF16 = mybir.dt.bfloat16
F32 = mybir.dt.float32

B, S, H = 2, 2048, 2048
NH, NKV, HD = 16, 4, 128
NCORES = 8
HPC = NH // NCORES          # q heads per core = 2
POS = B * S                 # 4096 flattened rows
RPC = POS // NCORES         # output rows per core = 512
KT = H // 128               # 16 contraction tiles for projections
PT_N = POS // 512           # 8 pos-tiles of 512
SCALE = 1.0 / math.sqrt(HD)

_CACHE = {}


def _build():
    nc = bacc.Bacc("TRN2", target_bir_lowering=False, debug=False,
                   num_devices=NCORES)

    xT = nc.declare_dram_parameter("xT", [H, POS], BF16, isOutput=False)
    wq = nc.declare_dram_parameter("wq", [H, HPC * HD], BF16, isOutput=False)
    wk = nc.declare_dram_parameter("wk", [H, HD], BF16, isOutput=False)
    wv = nc.declare_dram_parameter("wv", [H, HD], BF16, isOutput=False)
    cosT = nc.declare_dram_parameter("cosT", [HD, S], BF16, isOutput=False)
    ssinT = nc.declare_dram_parameter("ssinT", [HD, S], BF16, isOutput=False)
    wo = nc.declare_dram_parameter("wo", [NH * HD, H], BF16, isOutput=False)
    out = nc.declare_dram_parameter("out", [RPC, H], F32, isOutput=True)

    xT_t = xT.ap().rearrange("(k p) n -> p k n", p=128)
    wq_t = wq.ap().rearrange("(k p) m -> p k m", p=128)
    wk_t = wk.ap().rearrange("(k p) m -> p k m", p=128)
    wv_t = wv.ap().rearrange("(k p) m -> p k m", p=128)
    wo_t = wo.ap().rearrange("(k p) m -> p k m", p=128)

    with tile.TileContext(nc) as tc:
        with (
            tc.tile_pool(name="const", bufs=1) as const,
            tc.tile_pool(name="wpool", bufs=1) as wpool,
            tc.tile_pool(name="qkv", bufs=1) as qkv,
            tc.tile_pool(name="dram", bufs=1, space="DRAM") as dram,
        ):
            # ---- constants / weights resident in SBUF ----
            ident = const.tile([128, 128], BF16)
            make_identity(nc, ident)
            # lower-triangular 0/1 mask for the diagonal 128x128 block
            tri = const.tile([128, 128], BF16)
            nc.gpsimd.memset(tri, 1.0)
            nc.gpsimd.affine_select(
                out=tri, in_=tri, compare_op=mybir.AluOpType.is_ge,
                fill=0.0, base=0, pattern=[[-1, 128]], channel_multiplier=1,
            )  # where (p - c) >= 0 keep 1.0 (lower tri), else fill 0.0
            # upper-triangular (incl diag) mask: valid where kpos <= q
            triT = const.tile([128, 128], BF16)
            nc.gpsimd.memset(triT, 1.0)
            nc.gpsimd.affine_select(
                out=triT, in_=triT, compare_op=mybir.AluOpType.is_ge,
                fill=0.0, base=0, pattern=[[1, 128]], channel_multiplier=-1,
            )  # keep 1.0 where (c - p) >= 0, i.e. kpos <= q
            ones_sb = const.tile([128, 128], BF16)
            nc.gpsimd.memset(ones_sb, 1.0)

            cos_sb = const.tile([128, S], BF16)
            sin_sb = const.tile([128, S], BF16)
            cs_dmas = [nc.sync.dma_start(cos_sb[:], cosT.ap()),
                       nc.sync.dma_start(sin_sb[:], ssinT.ap())]

            wq_sb = wpool.tile([128, KT, HPC * HD], BF16)
            wk_sb = wpool.tile([128, KT, HD], BF16)
            wv_sb = wpool.tile([128, KT, HD], BF16)
            nc.sync.dma_start(wq_sb[:], wq_t)
            nc.sync.dma_start(wk_sb[:], wk_t)
            nc.sync.dma_start(wv_sb[:], wv_t)
            wo_sb = wpool.tile([128, KT, H], BF16)
            at1_sb = wpool.tile([128, NCORES, RPC], BF16)
            at2_sb = wpool.tile([128, NCORES, RPC], BF16)

            # persistent q/k/v for both batches (bf16)
            # qT: [d, pos] per head; kT: [d, pos]; v: [pos-tile, d]
            q_all = qkv.tile([128, HPC, POS], BF16)
            k_all = qkv.tile([128, POS], BF16)
            v_all = qkv.tile([128, POS // 128, HD], BF16)

            a2a_in1 = dram.tile([NCORES, HD, RPC], BF16)
            a2a_out1 = dram.tile([NCORES, HD, RPC], BF16)
            a2a_in2 = dram.tile([NCORES, HD, RPC], BF16)
            a2a_out2 = dram.tile([NCORES, HD, RPC], BF16)

            # ================= Phase 1: QKV projection + RoPE ============
            def rope(dst, ps, c0):
                """dst[128,512] bf16 = ps*cos + swap_halves(ps)*ssin."""
                ra = rope_pool.tile([128, 512], BF16, name="ra", tag="ra",
                                    bufs=3)
                rb = rope_pool.tile([128, 512], BF16, name="rb", tag="rb",
                                    bufs=3)
                nc.vector.tensor_tensor(
                    ra[:], ps[:], cos_sb[:, c0:c0 + 512], mybir.AluOpType.mult)
                nc.vector.tensor_tensor(
                    rb[0:64, :], ps[64:128, :], sin_sb[0:64, c0:c0 + 512],
                    mybir.AluOpType.mult)
                nc.vector.tensor_tensor(
                    rb[64:128, :], ps[0:64, :], sin_sb[64:128, c0:c0 + 512],
                    mybir.AluOpType.mult)
                nc.vector.tensor_tensor(dst, ra[:], rb[:],
                                        mybir.AluOpType.add)

            with (
                tc.tile_pool(name="xtiles", bufs=1) as xtiles,
                tc.tile_pool(name="rope_pool", bufs=1) as rope_pool,
                tc.tile_pool(name="ps1", bufs=1, space="PSUM") as ps1,
            ):
                for pt in range(PT_N):
                    c0 = (pt * 512) % S   # rope table column offset
                    x_t = xtiles.tile([128, KT, 512], BF16, name="x_t",
                                      tag="x", bufs=3)
                    for k4 in range(4):
                        xd = nc.sync.dma_start(
                            x_t[:, k4 * 4:(k4 + 1) * 4, :],
                            xT_t[:, k4 * 4:(k4 + 1) * 4,
                                 pt * 512:(pt + 1) * 512])

                    if pt == 0:
                        for csd in cs_dmas:
                            add_dep_helper(csd.ins, xd.ins, False)
                    for hh in range(HPC):
                        ps_q = ps1.tile([128, 512], F32, name="ps_q",
                                        tag="psq", bufs=2)
                        for k in range(KT):
                            nc.tensor.matmul(
                                ps_q[:], wq_sb[:, k, hh * 128:(hh + 1) * 128],
                                x_t[:, k, :], start=(k == 0), stop=(k == KT - 1))
                        rope(q_all[:, hh, pt * 512:(pt + 1) * 512], ps_q, c0)

                    ps_k = ps1.tile([128, 512], F32, name="ps_k", tag="psk",
                                    bufs=2)
                    for k in range(KT):
                        nc.tensor.matmul(ps_k[:], wk_sb[:, k, :], x_t[:, k, :],
                                         start=(k == 0), stop=(k == KT - 1))
                    rope(k_all[:, pt * 512:(pt + 1) * 512], ps_k, c0)

                    for m4 in range(4):
                        ps_v = ps1.tile([128, 128], F32, name="ps_v",
                                        tag="psv", bufs=3)
                        for k in range(KT):
                            nc.tensor.matmul(
                                ps_v[:], x_t[:, k, m4 * 128:(m4 + 1) * 128],
                                wv_sb[:, k, :], start=(k == 0),
                                stop=(k == KT - 1))
                        nc.scalar.copy(v_all[:, pt * 4 + m4, :], ps_v[:])

            # ================= Phase 2: causal flash attention ===========
            # ST layout: scores transposed [kpos, q]; exp writes P^T straight
            # to SBUF; denominators via ones-matmul (replicated across
            # partitions); PV consumes P^T directly. 512-query superblocks.
            with (
                tc.tile_pool(name="att", bufs=1) as att,
                tc.tile_pool(name="ps2", bufs=1, space="PSUM") as ps2,
            ):
                pending = []   # instructions to pull early in the schedule
                first_att_mm = [None]
                for hh in range(HPC):
                    a2a_in = a2a_in1 if hh == 0 else a2a_in2
                    a2a_out_h = a2a_out1 if hh == 0 else a2a_out2
                    for b in range(B):
                        qT = q_all[:, hh, b * S:(b + 1) * S]
                        kTb = k_all[:, b * S:(b + 1) * S]
                        voff = b * (S // 128)
                        for qsb in range(S // 512):
                            qs = qsb * 512
                            o_ps = ps2.tile([128, 512], F32, name="o_ps",
                                            tag="ops", bufs=3)
                            sum_ps = ps2.tile([128, 512], F32, name="sum_ps",
                                              tag="sums", bufs=3)
                            nkt = 4 * qsb + 4
                            for kt in range(nkt):
                                jj = kt - 4 * qsb   # >=0 on the diagonal
                                c0 = 0 if jj < 0 else jj * 128
                                st_ps = ps2.tile([128, 512], F32, name="st_ps",
                                                 tag="stps", bufs=2)
                                mm = nc.tensor.matmul(
                                    st_ps[:, c0:512],
                                    kTb[:, kt * 128:(kt + 1) * 128],
                                    qT[:, qs + c0:qs + 512],
                                    start=True, stop=True)
                                if first_att_mm[0] is None:
                                    first_att_mm[0] = mm
                                for pend in pending:
                                    add_dep_helper(mm.ins, pend.ins, False)
                                pending = []
                                pt_sb = att.tile([128, 512], BF16,
                                                 name="pt_sb", tag="pt",
                                                 bufs=4)
                                nc.scalar.activation(
                                    pt_sb[:, c0:512], st_ps[:, c0:512],
                                    mybir.ActivationFunctionType.Exp,
                                    scale=SCALE)
                                if jj >= 0:
                                    nc.vector.tensor_tensor(
                                        pt_sb[:, jj * 128:(jj + 1) * 128],
                                        pt_sb[:, jj * 128:(jj + 1) * 128],
                                        triT[:], mybir.AluOpType.mult)
                                nc.tensor.matmul(
                                    sum_ps[:, c0:512], ones_sb[:],
                                    pt_sb[:, c0:512],
                                    start=(kt == 0), stop=(kt == nkt - 1))
                                nc.tensor.matmul(
                                    o_ps[:, c0:512], v_all[:, voff + kt, :],
                                    pt_sb[:, c0:512],
                                    start=(kt == 0), stop=(kt == nkt - 1))

                            recip = att.tile([128, 512], F32, name="recip",
                                             tag="recip", bufs=2)
                            nc.vector.reciprocal_approx_fast(recip[:],
                                                             sum_ps[:])
                            oT_sb = att.tile([128, 512], BF16, name="oT_sb",
                                             tag="osb", bufs=2)
                            nc.vector.scalar_tensor_tensor(
                                oT_sb[:], o_ps[:], 1.0, recip[:],
                                mybir.AluOpType.mult, mybir.AluOpType.mult)
                            j = b * 4 + qsb
                            d = nc.sync.dma_start(a2a_in[j, :, :], oT_sb[:])
                            pending.append(d)
                    cc = nc.gpsimd.collective_compute(
                        "AllToAll", mybir.AluOpType.bypass,
                        replica_groups=[list(range(NCORES))],
                        ins=[a2a_in.opt()], outs=[a2a_out_h.opt()])
                    pending.append(cc)

            # ================= Phase 3: o_proj (two-part accumulate) =====
            with (
                tc.tile_pool(name="proj", bufs=1) as proj,
                tc.tile_pool(name="ps3", bufs=1, space="PSUM") as ps3,
            ):
                for k4 in range(4):
                    wd = nc.sync.dma_start(
                        wo_sb[:, k4 * 4:(k4 + 1) * 4, :],
                        wo_t[:, k4 * 4:(k4 + 1) * 4, :])
                    add_dep_helper(wd.ins, first_att_mm[0].ins, False)
                for r in range(NCORES):
                    nc.sync.dma_start(at1_sb[:, r, :], a2a_out1[r, :, :])
                for r in range(NCORES):
                    nc.sync.dma_start(at2_sb[:, r, :], a2a_out2[r, :, :])
                s1_sb = proj.tile([128, 16, 512], F32)
                for mp in range(RPC // 128):
                    for nn in range(H // 512):
                        ti = mp * 4 + nn
                        ps_a = ps3.tile([128, 512], F32, name="ps_a",
                                        tag="po", bufs=4)
                        for r in range(NCORES):
                            nc.tensor.matmul(
                                ps_a[:], at1_sb[:, r, mp * 128:(mp + 1) * 128],
                                wo_sb[:, 2 * r, nn * 512:(nn + 1) * 512],
                                start=(r == 0), stop=(r == NCORES - 1))
                        nc.scalar.copy(s1_sb[:, ti, :], ps_a[:])
                for mp in range(RPC // 128):
                    for nn in range(H // 512):
                        ti = mp * 4 + nn
                        ps_b = ps3.tile([128, 512], F32, name="ps_b",
                                        tag="po", bufs=4)
                        for r in range(NCORES):
                            nc.tensor.matmul(
                                ps_b[:], at2_sb[:, r, mp * 128:(mp + 1) * 128],
                                wo_sb[:, 2 * r + 1, nn * 512:(nn + 1) * 512],
                                start=(r == 0), stop=(r == NCORES - 1))
                        ev = proj.tile([128, 512], F32, name="ev", tag="ev",
                                       bufs=3)
                        nc.vector.scalar_tensor_tensor(
                            ev[:], ps_b[:], 1.0, s1_sb[:, ti, :],
                            mybir.AluOpType.mult, mybir.AluOpType.add)
                        nc.sync.dma_start(
                            out.ap()[mp * 128:(mp + 1) * 128,
                                     nn * 512:(nn + 1) * 512], ev[:])

    nc.compile()
    return nc


def _get_nc():
    if "nc" not in _CACHE:
        _CACHE["nc"] = _build()
    return _CACHE["nc"]


def _prep_inputs(x, cos, sin, wq, wk, wv, wo):
    bf = ml_dtypes.bfloat16
    xT = np.ascontiguousarray(
        np.asarray(x, np.float32).reshape(POS, H).T).astype(bf)
    cosT = np.ascontiguousarray(np.asarray(cos, np.float32).T).astype(bf)
    sinT = np.asarray(sin, np.float32).T.copy()
    sinT[0:64, :] = -sinT[0:64, :]
    sinT = np.ascontiguousarray(sinT).astype(bf)
    wo_b = np.asarray(wo, np.float32).astype(bf)
    wq = np.asarray(wq, np.float32)
    wk = np.asarray(wk, np.float32)
    wv = np.asarray(wv, np.float32)

    in_maps = []
    for i in range(NCORES):
        kv = i // 2
        in_maps.append({
            "xT": xT,
            "wq": np.ascontiguousarray(
                wq[:, i * HPC * HD:(i + 1) * HPC * HD]).astype(bf),
            "wk": np.ascontiguousarray(
                wk[:, kv * HD:(kv + 1) * HD]).astype(bf),
            "wv": np.ascontiguousarray(
                wv[:, kv * HD:(kv + 1) * HD]).astype(bf),
            "cosT": cosT,
            "ssinT": sinT,
            "wo": wo_b,
        })
    return in_maps


def kernel(x, cos, sin, wq, wk, wv, wo, _trace=False):
    nc = _get_nc()
    in_maps = _prep_inputs(x, cos, sin, wq, wk, wv, wo)
    res = run_bass_kernel_spmd(nc, in_maps, core_ids=list(range(NCORES)),
                               trace=_trace)
    rows = np.concatenate([np.asarray(res.results[i]["out"])
                           for i in range(NCORES)], axis=0)
    out = rows.reshape(B, S, H).astype(np.float32)
    if _trace:
        _CACHE["last_exec_time_ns"] = res.exec_time_ns
        _CACHE["last_results"] = res
    return out
